# revision 1
# baseline (speedup 1.0000x reference)
"""ChebConv (K=2) + temporal Conv1d GNN kernel for 8 Trainium2 NeuronCores.

Strategy (data-parallel over destination nodes):
  - Node axis padded to 50176 = 392 blocks of 128; core c owns blocks
    [49c, 49c+49).
  - Host precomputes w_hat (edge weights of -D^-1/2 A D^-1/2) and sorts the
    edge list by (dst block, src half, dst subblock-of-32), padding each
    group to a multiple of 128 so all 8 cores share one static program.
  - Per block, the device gathers x rows of the edges' sources from an
    fp16 node-major copy of x via SWDGE dma_gather (two calls: src halves,
    since gather indices are int16), builds a sparse "one-hot * w_hat"
    matrix on the fly with broadcast-AP is_equal/mult, and reduces the
    messages with TensorE matmuls into PSUM (segment-sum as matmul).
  - The Chebyshev combine + temporal conv collapse into dense per-node
    matmuls with host-prefolded weights; LeakyReLU finishes on-chip.
"""

import numpy as np

N = 50000
E = 1600000
W = 12
C = 32
WC = W * C            # 384
NCORES = 8
P = 128
NPAD = 50176          # 392 * 128
NB = NPAD // P        # 392
SLOTS = NB // NCORES  # 49
HALF = NPAD // 2      # 25088
NSB = 4               # dst subblocks of 32 per block

_cache = {}


def _host_prep(x, A, Ew):
    src = np.asarray(A[0], np.int64)
    dst = np.asarray(A[1], np.int64)
    Ew = np.asarray(Ew, np.float32)

    deg = np.bincount(dst, weights=Ew.astype(np.float64), minlength=N).astype(np.float32)
    dinv = np.where(deg > 0, 1.0 / np.sqrt(np.maximum(deg, 1e-12)), 0.0).astype(np.float32)
    w_hat = (-dinv[src] * Ew * dinv[dst]).astype(np.float32)

    # node-major x: [NPAD, W*C]
    xrow = np.zeros((NPAD, WC), np.float32)
    xrow[:N] = np.asarray(x, np.float32).transpose(1, 0, 2).reshape(N, WC)
    xrow16 = xrow.astype(np.float16)

    blk = dst >> 7
    sb = (dst >> 5) & 3
    hh = (src >= HALF).astype(np.int64)
    gid = (blk * 2 + hh) * 4 + sb
    order = np.argsort(gid, kind="stable")
    g_sorted = gid[order]
    src_s = src[order]
    dstl_s = (dst[order] & 31).astype(np.float16)
    what_s = w_hat[order].astype(np.float16)
    counts = np.bincount(gid, minlength=NB * 8).reshape(NB, 2, 4)
    gstart = np.zeros(NB * 8 + 1, np.int64)
    np.cumsum(counts.reshape(-1), out=gstart[1:])

    # static chunk counts per (slot, h, s): max over cores
    cnt_c = counts.reshape(NCORES, SLOTS, 2, 4)
    Kg = np.maximum(1, -(-cnt_c // 128)).max(axis=0)  # [SLOTS, 2, 4]
    Jh = Kg.sum(axis=2)                               # [SLOTS, 2]
    Ji = Jh.sum(axis=1)                               # [SLOTS]
    JT = int(Ji.sum())
    IWT = JT * 8

    # column offsets
    joff = np.zeros(SLOTS + 1, np.int64)
    np.cumsum(Ji, out=joff[1:])
    ioff = joff * 8

    idx16 = np.zeros((NCORES, 128, IWT), np.int16)
    dstl_t = np.zeros((NCORES, 128, JT), np.float16)
    what_t = np.zeros((NCORES, 128, JT), np.float16)
    xslot = np.zeros((NCORES, SLOTS * P, WC), np.float32)

    for c in range(NCORES):
        xslot[c] = xrow[c * SLOTS * P:(c + 1) * SLOTS * P]
        for i in range(SLOTS):
            b = c * SLOTS + i
            for h in range(2):
                L = int(Jh[i, h]) * 128
                V = np.zeros(L, np.int16)
                D = np.zeros(L, np.float16)
                Wv = np.zeros(L, np.float16)
                base = 0
                for s in range(4):
                    g = (b * 2 + h) * 4 + s
                    n = int(gstart[g + 1] - gstart[g])
                    sl = slice(int(gstart[g]), int(gstart[g] + n))
                    V[base:base + n] = (src_s[sl] - h * HALF).astype(np.int16)
                    D[base:base + n] = dstl_s[sl]
                    Wv[base:base + n] = what_s[sl]
                    base += int(Kg[i, h, s]) * 128
                co = int(joff[i] + (Jh[i, 0] if h else 0))
                idx_blk = V.reshape(-1, 16).T                    # [16, L/16]
                idx16[c, :, co * 8: co * 8 + L // 16] = np.tile(idx_blk, (8, 1))
                dstl_t[c, :, co: co + L // 128] = D.reshape(-1, 128).T
                what_t[c, :, co: co + L // 128] = Wv.reshape(-1, 128).T

    return xrow16, xslot, idx16, dstl_t, what_t, Kg, Jh, Ji, joff, JT, IWT


def _fold_weights(Wcheb, bcheb, Wconv, bconv):
    Wcheb = np.asarray(Wcheb, np.float32)
    bcheb = np.asarray(bcheb, np.float32)
    Wconv = np.asarray(Wconv, np.float32)
    bconv = np.asarray(bconv, np.float32)
    # pairs (path, gi, go) with |gi-go|<=1
    pairs = []
    for go in range(3):
        for gi in range(max(0, go - 1), min(3, go + 2)):
            for path in range(2):
                pairs.append((path, gi, go))
    mats = np.zeros((len(pairs), 128, 128), np.float32)
    for pi, (path, gi, go) in enumerate(pairs):
        for wo in range(4 * go, 4 * go + 4):
            for k in range(3):
                wi = wo + k - 1
                if not (4 * gi <= wi < 4 * gi + 4) or not (0 <= wi < W):
                    continue
                Cmat = Wcheb[wi, path] @ Wconv[:, :, k].T  # [ci, co]
                r0 = 32 * (wi - 4 * gi)
                c0 = 32 * (wo - 4 * go)
                mats[pi, r0:r0 + 32, c0:c0 + 32] = Cmat
    mats_sb = np.ascontiguousarray(mats.transpose(1, 0, 2).reshape(128, -1))
    bias = np.zeros((12, 32), np.float32)
    for wo in range(12):
        bias[wo] = bconv.copy()
        for k in range(3):
            wi = wo + k - 1
            if 0 <= wi < W:
                bias[wo] += bcheb[wi] @ Wconv[:, :, k].T
    bias_sb = bias.reshape(3, 128).T.copy()  # [128, 3]
    return mats_sb, bias_sb, pairs


def _build_program(Kg, Jh, Ji, joff, JT, IWT, n_pairs):
    import concourse.bacc as bacc
    import concourse.tile as tile
    from concourse import mybir
    import concourse.bass as bass  # noqa

    nc = bacc.Bacc("TRN2", target_bir_lowering=False, debug=False,
                   num_devices=NCORES)
    f16, f32, i16 = mybir.dt.float16, mybir.dt.float32, mybir.dt.int16
    xrow16 = nc.dram_tensor("xrow16", [NPAD, WC], f16, kind="ExternalInput")
    xslot = nc.dram_tensor("xslot", [SLOTS * P, WC], f32, kind="ExternalInput")
    idx16 = nc.dram_tensor("idx16", [128, IWT], i16, kind="ExternalInput")
    dstl = nc.dram_tensor("dstl", [128, JT], f16, kind="ExternalInput")
    what = nc.dram_tensor("what", [128, JT], f16, kind="ExternalInput")
    mats = nc.dram_tensor("mats", [128, n_pairs * 128], f32, kind="ExternalInput")
    biasd = nc.dram_tensor("biasd", [128, 3], f32, kind="ExternalInput")
    iota = nc.dram_tensor("iota", [128, 32], f16, kind="ExternalInput")
    ident = nc.dram_tensor("ident", [128, 128], f32, kind="ExternalInput")
    out_pc = nc.dram_tensor("out_pc", [SLOTS * P, WC], f32, kind="ExternalOutput")

    pairs_by_go = [[], [], []]
    pi = 0
    for go in range(3):
        for gi in range(max(0, go - 1), min(3, go + 2)):
            for path in range(2):
                pairs_by_go[go].append((pi, gi, path))
                pi += 1

    with tile.TileContext(nc) as tc:
        with tc.tile_pool(name="const", bufs=1) as cp, \
             tc.tile_pool(name="sb", bufs=2) as sb, \
             tc.tile_pool(name="xgp", bufs=2) as xgp, \
             tc.tile_pool(name="pst1", bufs=2, space="PSUM") as pst1, \
             tc.tile_pool(name="pstr", bufs=2, space="PSUM") as pstr, \
             tc.tile_pool(name="psy", bufs=2, space="PSUM") as psy:
            mats_t = cp.tile([128, n_pairs * 128], f32)
            nc.sync.dma_start(out=mats_t[:], in_=mats.ap())
            bias_t = cp.tile([128, 3], f32)
            nc.sync.dma_start(out=bias_t[:], in_=biasd.ap())
            iota_t = cp.tile([128, 32], f16)
            nc.sync.dma_start(out=iota_t[:], in_=iota.ap())
            id_t = cp.tile([128, 128], f32)
            nc.sync.dma_start(out=id_t[:], in_=ident.ap())

            import os
            nslots = int(os.environ.get("K_SLOTS", SLOTS))
            sp_flag = os.environ.get("K_SINGLE_PACKET", "0") == "1"
            JMAX = int(Ji.max())
            for i in range(nslots):
                J0, J1 = int(Jh[i, 0]), int(Jh[i, 1])
                J = J0 + J1
                jo = int(joff[i])

                idx_t = sb.tile([128, JMAX * 8], i16, tag="idx")
                nc.sync.dma_start(out=idx_t[:, :J * 8],
                                  in_=idx16.ap()[:, jo * 8:(jo + J) * 8])
                dm_t = sb.tile([128, JMAX], f16, tag="dm")
                nc.sync.dma_start(out=dm_t[:, :J], in_=dstl.ap()[:, jo:jo + J])
                wh_t = sb.tile([128, JMAX], f16, tag="wh")
                nc.sync.dma_start(out=wh_t[:, :J], in_=what.ap()[:, jo:jo + J])

                xg = xgp.tile([128, JMAX, WC], f16, tag="xg")
                nc.gpsimd.dma_gather(
                    xg[:, 0:J0, :], xrow16.ap()[0:HALF, :],
                    idx_t[:, 0:J0 * 8], J0 * 128, J0 * 128, WC,
                    single_packet=sp_flag)
                nc.gpsimd.dma_gather(
                    xg[:, J0:J, :], xrow16.ap()[HALF:NPAD, :],
                    idx_t[:, J0 * 8:J * 8], J1 * 128, J1 * 128, WC,
                    single_packet=sp_flag)

                eq = sb.tile([128, JMAX, 32], f16, tag="eq")
                nc.vector.tensor_tensor(
                    out=eq[:, :J, :],
                    in0=dm_t[:, :J].unsqueeze(2).to_broadcast([128, J, 32]),
                    in1=iota_t[:].unsqueeze(1).to_broadcast([128, J, 32]),
                    op=mybir.AluOpType.is_equal)
                wm = sb.tile([128, JMAX, 32], f16, tag="wm")
                nc.vector.tensor_tensor(
                    out=wm[:, :J, :],
                    in0=eq[:, :J, :],
                    in1=wh_t[:, :J].unsqueeze(2).to_broadcast([128, J, 32]),
                    op=mybir.AluOpType.mult)

                psum_t1 = pst1.tile([128, WC], f32, space="PSUM", tag="t1")
                for s in range(4):
                    first = True
                    for h in range(2):
                        off = (0 if h == 0 else J0) + int(Kg[i, h, :s].sum())
                        for cidx in range(int(Kg[i, h, s])):
                            j = off + cidx
                            last = (h == 1 and cidx == int(Kg[i, 1, s]) - 1)
                            nc.tensor.matmul(
                                out=psum_t1[32 * s:32 * s + 32, :],
                                lhsT=wm[:, j:j + 1, :],
                                rhs=xg[:, j:j + 1, :],
                                start=first, stop=last,
                                tile_position=(0, 32 * s))
                            first = False

                t1sb = sb.tile([128, WC], f32, tag="t1sb")
                nc.scalar.copy(out=t1sb[:], in_=psum_t1[:])
                xb = sb.tile([128, WC], f32, tag="xb")
                nc.sync.dma_start(out=xb[:], in_=xslot.ap()[i * P:(i + 1) * P, :])

                xt = sb.tile([128, WC], f32, tag="xt")
                t1t = sb.tile([128, WC], f32, tag="t1t")
                for t in range(3):
                    ptr = pstr.tile([128, 128], f32, space="PSUM", tag="tr")
                    nc.tensor.transpose(out=ptr[:], in_=xb[:, 128 * t:128 * t + 128],
                                        identity=id_t[:])
                    nc.vector.tensor_copy(out=xt[:, 128 * t:128 * t + 128], in_=ptr[:])
                    ptr2 = pstr.tile([128, 128], f32, space="PSUM", tag="tr")
                    nc.tensor.transpose(out=ptr2[:], in_=t1sb[:, 128 * t:128 * t + 128],
                                        identity=id_t[:])
                    nc.scalar.copy(out=t1t[:, 128 * t:128 * t + 128], in_=ptr2[:])

                yo = sb.tile([128, WC], f32, tag="yo")
                osb = sb.tile([128, WC], f32, tag="osb")
                for go in range(3):
                    py = psy.tile([128, 128], f32, space="PSUM", tag="y")
                    plist = pairs_by_go[go]
                    for n_, (pi_, gi, path) in enumerate(plist):
                        rhs = (xt if path == 0 else t1t)[:, 128 * gi:128 * gi + 128]
                        nc.tensor.matmul(
                            out=py[:], lhsT=mats_t[:, 128 * pi_:128 * pi_ + 128],
                            rhs=rhs, start=(n_ == 0), stop=(n_ == len(plist) - 1),
                            tile_position=(0, 0))
                    ysl = yo[:, 128 * go:128 * go + 128]
                    nc.scalar.activation(out=ysl, in_=py[:],
                                         func=mybir.ActivationFunctionType.Identity,
                                         bias=bias_t[:, go:go + 1], scale=1.0)
                    tl = sb.tile([128, 128], f32, tag="tl")
                    nc.vector.tensor_scalar_mul(out=tl[:], in0=ysl, scalar1=0.01)
                    nc.vector.tensor_tensor(out=ysl, in0=ysl, in1=tl[:],
                                            op=mybir.AluOpType.max)
                    ptr3 = pstr.tile([128, 128], f32, space="PSUM", tag="tr")
                    nc.tensor.transpose(out=ptr3[:], in_=ysl, identity=id_t[:])
                    nc.vector.tensor_copy(out=osb[:, 128 * go:128 * go + 128],
                                          in_=ptr3[:])
                nc.sync.dma_start(out=out_pc.ap()[i * P:(i + 1) * P, :], in_=osb[:])

    nc.compile()
    return nc


def kernel(x, A, Ew, Wcheb, bcheb, Wconv, bconv, batch_size=1):
    from concourse.bass_utils import run_bass_kernel_spmd

    xrow16, xslot, idx16, dstl_t, what_t, Kg, Jh, Ji, joff, JT, IWT = \
        _host_prep(x, A, Ew)
    mats_sb, bias_sb, pairs = _fold_weights(Wcheb, bcheb, Wconv, bconv)

    key = (JT, IWT, tuple(Ji.tolist()))
    if key not in _cache:
        _cache[key] = _build_program(Kg, Jh, Ji, joff, JT, IWT, len(pairs))
    nc = _cache[key]

    iota_np = np.tile(np.arange(32, dtype=np.float16)[None, :], (128, 1))
    ident_np = np.eye(128, dtype=np.float32)
    in_maps = []
    for c in range(NCORES):
        in_maps.append(dict(
            xrow16=xrow16, xslot=xslot[c], idx16=idx16[c],
            dstl=dstl_t[c], what=what_t[c], mats=mats_sb, biasd=bias_sb,
            iota=iota_np, ident=ident_np))
    res = run_bass_kernel_spmd(nc, in_maps, core_ids=list(range(NCORES)))
    full = np.concatenate([res.results[c]["out_pc"] for c in range(NCORES)], axis=0)
    return np.ascontiguousarray(full[:N]).reshape(N, W, C).astype(np.float32)



# revision 3
# speedup vs baseline: 1.5873x; 1.5873x over previous
"""ChebConv (K=2) + temporal Conv1d GNN kernel for 8 Trainium2 NeuronCores.

Strategy (data-parallel over destination nodes):
  - Node axis padded to 50176 = 392 blocks of 128; blocks are assigned to
    (core, slot) pairs by a balance heuristic so all 8 cores share one
    static program with minimal padding.
  - Host precomputes w_hat (edge weights of -D^-1/2 A D^-1/2) and sorts the
    edge list by (dst block, src half, dst subblock-of-32); groups are
    padded to 16-edge granularity to the max count across cores.
  - Per block, the device gathers x rows of the edges' sources from a
    node-major copy of x via SWDGE dma_gather (fp8 rows padded to 512 B for
    full-rate DMA descriptors, or fp16 768 B rows), builds a sparse
    "one-hot * w_hat" matrix with broadcast-AP is_equal/mult, and reduces
    messages with TensorE matmuls in SWAPPED orientation
    (lhsT = gathered x, rhs = one-hot) so T1 lands feature-major in PSUM.
  - The Chebyshev combine + temporal conv collapse into dense per-node
    matmuls with host-prefolded fp16 weights, all in the transposed
    [feature, node] layout; LeakyReLU on-chip; output written transposed
    and unshuffled on the host.
"""

import numpy as np
import ml_dtypes

N = 50000
E = 1600000
W = 12
C = 32
WC = W * C            # 384
NCORES = 8
P = 128
NPAD = 50176          # 392 * 128
NB = NPAD // P        # 392
SLOTS = NB // NCORES  # 49
HALF = NPAD // 2      # 25088

G8 = True             # gather in fp8 (512B padded rows) vs fp16 (768B rows)
ELEM8 = 512

_cache = {}


def _plan_edges(src, dst):
    """Shared static plan + per-core edge data layouts."""
    blk = dst >> 7
    hh = (src >= HALF).astype(np.int64)
    sb = (dst >> 5) & 3
    gid = (blk * 2 + hh) * 4 + sb
    order = np.argsort(gid, kind="stable")
    src_s = src[order]
    dstl_s = (dst[order] & 127).astype(np.int64)
    counts = np.bincount(gid, minlength=NB * 8).reshape(NB, 2, 4)
    gstart = np.zeros(NB * 8 + 1, np.int64)
    np.cumsum(counts.reshape(-1), out=gstart[1:])

    # block -> (core, slot): snake by total count for balance
    tot = counts.sum(axis=(1, 2))
    bo = np.argsort(-tot, kind="stable")
    Bmap = np.zeros((NCORES, SLOTS), np.int64)
    for i in range(SLOTS):
        for c in range(NCORES):
            Bmap[c, i] = bo[i * NCORES + c]

    # per (slot, h, s): padded count M = 16*ceil(max_core cnt / 16), >= 16
    M = np.zeros((SLOTS, 2, 4), np.int64)
    for i in range(SLOTS):
        cnt = counts[Bmap[:, i]]          # [NCORES, 2, 4]
        M[i] = np.maximum(16, 16 * ((cnt.max(axis=0) + 15) // 16))
    NI = M.sum(axis=2)                    # [SLOTS, 2] num_idxs per h
    Jc = -(-NI // 128)                    # chunks per h

    # matmul stream per slot: list of (col, chunk_j, s) + start/stop per s
    streams = []
    JX = np.zeros(SLOTS, np.int64)
    for i in range(SLOTS):
        ents = []
        for h in range(2):
            jbase = 0 if h == 0 else int(Jc[i, 0])
            e0 = 0
            for s in range(4):
                e1 = e0 + int(M[i, h, s])
                jlo, jhi = e0 // 128, -(-e1 // 128)
                for j in range(jlo, jhi):
                    ents.append((jbase + j, s))
                e0 = e1
        first = {}
        last = {}
        for ci, (j, s) in enumerate(ents):
            first.setdefault(s, ci)
            last[s] = ci
        stream = [(ci, j, s, first[s] == ci, last[s] == ci)
                  for ci, (j, s) in enumerate(ents)]
        streams.append(stream)
        JX[i] = len(ents)

    iw = (NI[:, 0] + NI[:, 1]) // 16      # idx cols per slot
    iw_off = np.zeros(SLOTS + 1, np.int64)
    np.cumsum(iw, out=iw_off[1:])
    jx_off = np.zeros(SLOTS + 1, np.int64)
    np.cumsum(JX, out=jx_off[1:])

    return dict(order=order, src_s=src_s, dstl_s=dstl_s, gstart=gstart,
                Bmap=Bmap, M=M, NI=NI, Jc=Jc, streams=streams, JX=JX,
                iw=iw, iw_off=iw_off, jx_off=jx_off)


def _host_prep(x, A, Ew):
    src = np.asarray(A[0], np.int64)
    dst = np.asarray(A[1], np.int64)
    Ew = np.asarray(Ew, np.float32)

    deg = np.bincount(dst, weights=Ew.astype(np.float64), minlength=N).astype(np.float32)
    dinv = np.where(deg > 0, 1.0 / np.sqrt(np.maximum(deg, 1e-12)), 0.0).astype(np.float32)
    w_hat = (-dinv[src] * Ew * dinv[dst]).astype(np.float32)

    xrow = np.zeros((NPAD, WC), np.float32)
    xrow[:N] = np.asarray(x, np.float32).transpose(1, 0, 2).reshape(N, WC)
    xrow16 = xrow.astype(np.float16)
    xrow8 = None
    if G8:
        xrow8 = np.zeros((NPAD, ELEM8), ml_dtypes.float8_e4m3fn)
        xrow8[:, :WC] = xrow.astype(ml_dtypes.float8_e4m3fn)

    plan = _plan_edges(src, dst)
    what_s = w_hat[plan["order"]].astype(np.float16)
    src_s, dstl_s, gstart = plan["src_s"], plan["dstl_s"], plan["gstart"]
    M, NI, Jc, Bmap = plan["M"], plan["NI"], plan["Jc"], plan["Bmap"]
    streams, JX = plan["streams"], plan["JX"]
    iw_off, jx_off = plan["iw_off"], plan["jx_off"]

    IWT = int(iw_off[-1])
    JXT = int(jx_off[-1])
    idx16 = np.zeros((NCORES, 128, IWT), np.int16)
    dmwh = np.zeros((NCORES, 128, 2 * JXT), np.float16)
    xbs = np.zeros((NCORES, SLOTS * P, WC), np.float16)

    for c in range(NCORES):
        for i in range(SLOTS):
            b = int(Bmap[c, i])
            xbs[c, i * P:(i + 1) * P] = xrow16[b * P:(b + 1) * P]
            JcT = int(Jc[i, 0] + Jc[i, 1])
            Dch = np.full(JcT * 128, 255, np.float16)   # dst&127 per chunk pos
            Wch = np.zeros(JcT * 128, np.float16)
            icol = int(iw_off[i])
            for h in range(2):
                L = int(NI[i, h])
                V = np.zeros(L, np.int16)
                cbase = 0 if h == 0 else int(Jc[i, 0]) * 128
                e0 = 0
                for s in range(4):
                    g = (b * 2 + h) * 4 + s
                    n = int(gstart[g + 1] - gstart[g])
                    sl = slice(int(gstart[g]), int(gstart[g] + n))
                    V[e0:e0 + n] = (src_s[sl] - h * HALF).astype(np.int16)
                    Dch[cbase + e0:cbase + e0 + n] = dstl_s[sl]
                    Wch[cbase + e0:cbase + e0 + n] = what_s[sl]
                    e0 += int(M[i, h, s])
                idx_blk = V.reshape(-1, 16).T               # [16, L/16]
                idx16[c, :, icol:icol + L // 16] = np.tile(idx_blk, (8, 1))
                icol += L // 16
            # dup-expanded dm/wh columns
            co = int(jx_off[i])
            jx = int(JX[i])
            for (ci, j, s, st, sp) in streams[i]:
                dmwh[c, :, 2 * co + ci] = Dch[j * 128:(j + 1) * 128]
                dmwh[c, :, 2 * co + jx + ci] = Wch[j * 128:(j + 1) * 128]

    return dict(xrow16=xrow16, xrow8=xrow8, idx16=idx16, dmwh=dmwh, xbs=xbs,
                plan=plan, IWT=IWT, JXT=JXT)


def _fold_weights(Wcheb, bcheb, Wconv, bconv):
    Wcheb = np.asarray(Wcheb, np.float32)
    bcheb = np.asarray(bcheb, np.float32)
    Wconv = np.asarray(Wconv, np.float32)
    bconv = np.asarray(bconv, np.float32)
    pairs = []
    for go in range(3):
        for gi in range(max(0, go - 1), min(3, go + 2)):
            for path in range(2):
                pairs.append((path, gi, go))
    mats = np.zeros((len(pairs), 128, 128), np.float32)
    for pi, (path, gi, go) in enumerate(pairs):
        for wo in range(4 * go, 4 * go + 4):
            for k in range(3):
                wi = wo + k - 1
                if not (4 * gi <= wi < 4 * gi + 4) or not (0 <= wi < W):
                    continue
                Cmat = Wcheb[wi, path] @ Wconv[:, :, k].T  # [ci, co]
                r0 = 32 * (wi - 4 * gi)
                c0 = 32 * (wo - 4 * go)
                mats[pi, r0:r0 + 32, c0:c0 + 32] = Cmat
    mats_sb = np.ascontiguousarray(
        mats.transpose(1, 0, 2).reshape(128, -1)).astype(np.float16)
    bias = np.zeros((12, 32), np.float32)
    for wo in range(12):
        bias[wo] = bconv.copy()
        for k in range(3):
            wi = wo + k - 1
            if 0 <= wi < W:
                bias[wo] += bcheb[wi] @ Wconv[:, :, k].T
    bias_sb = bias.reshape(3, 128).T.copy()  # [128, 3] fp32
    return mats_sb, bias_sb, pairs


def _build_program(plan, IWT, JXT, n_pairs):
    import concourse.bacc as bacc
    import concourse.tile as tile
    from concourse import mybir
    import concourse.bass as bass  # noqa

    M, NI, Jc = plan["M"], plan["NI"], plan["Jc"]
    streams, JX = plan["streams"], plan["JX"]
    iw_off, jx_off = plan["iw_off"], plan["jx_off"]
    JCmax = int((Jc[:, 0] + Jc[:, 1]).max())
    JXmax = int(JX.max())
    IWmax = int(((NI[:, 0] + NI[:, 1]) // 16).max())
    ELEM = ELEM8 if G8 else WC

    nc = bacc.Bacc("TRN2", target_bir_lowering=False, debug=False,
                   num_devices=NCORES)
    f16, f32, i16 = mybir.dt.float16, mybir.dt.float32, mybir.dt.int16
    f8 = mybir.dt.float8e4
    gdt = f8 if G8 else f16

    xrowg = nc.dram_tensor("xrowg", [NPAD, ELEM], gdt, kind="ExternalInput")
    xbs = nc.dram_tensor("xbs", [SLOTS * P, WC], f16, kind="ExternalInput")
    idx16 = nc.dram_tensor("idx16", [128, IWT], i16, kind="ExternalInput")
    dmwh = nc.dram_tensor("dmwh", [128, 2 * JXT], f16, kind="ExternalInput")
    mats = nc.dram_tensor("mats", [128, n_pairs * 128], f16, kind="ExternalInput")
    biasd = nc.dram_tensor("biasd", [128, 3], f32, kind="ExternalInput")
    iota4 = nc.dram_tensor("iota4", [128, 128], f16, kind="ExternalInput")
    ident = nc.dram_tensor("ident", [128, 128], f16, kind="ExternalInput")
    out_pc = nc.dram_tensor("out_pc", [128, 3, SLOTS * P], f16,
                            kind="ExternalOutput")

    pairs_by_go = [[], [], []]
    pi = 0
    for go in range(3):
        for gi in range(max(0, go - 1), min(3, go + 2)):
            for path in range(2):
                pairs_by_go[go].append((pi, gi, path))
                pi += 1

    with tile.TileContext(nc) as tc:
        with tc.tile_pool(name="const", bufs=1) as cp, \
             tc.tile_pool(name="sb", bufs=2) as sb, \
             tc.tile_pool(name="xgp", bufs=2) as xgp, \
             tc.tile_pool(name="osbp", bufs=2) as osbp, \
             tc.tile_pool(name="pst0", bufs=2, space="PSUM") as pst0, \
             tc.tile_pool(name="pst1", bufs=2, space="PSUM") as pst1, \
             tc.tile_pool(name="psy", bufs=2, space="PSUM") as psy:
            mats_t = cp.tile([128, n_pairs * 128], f16)
            nc.sync.dma_start(out=mats_t[:], in_=mats.ap())
            bias_t = cp.tile([128, 3], f32)
            nc.sync.dma_start(out=bias_t[:], in_=biasd.ap())
            iota_t = cp.tile([128, 128], f16)
            nc.sync.dma_start(out=iota_t[:], in_=iota4.ap())
            id_t = cp.tile([128, 128], f16)
            nc.sync.dma_start(out=id_t[:], in_=ident.ap())

            osb = None
            for i in range(SLOTS):
                NI0, NI1 = int(NI[i, 0]), int(NI[i, 1])
                Jc0, Jc1 = int(Jc[i, 0]), int(Jc[i, 1])
                jx = int(JX[i])
                io = int(iw_off[i])
                co = int(jx_off[i])
                niw = (NI0 + NI1) // 16

                idx_t = sb.tile([128, IWmax], i16, tag="idx")
                nc.sync.dma_start(out=idx_t[:, :niw],
                                  in_=idx16.ap()[:, io:io + niw])
                dmw_t = sb.tile([128, 2 * JXmax], f16, tag="dmw")
                nc.sync.dma_start(out=dmw_t[:, :2 * jx],
                                  in_=dmwh.ap()[:, 2 * co:2 * co + 2 * jx])

                xg = xgp.tile([128, JCmax, ELEM], gdt, tag="xg")
                nc.gpsimd.dma_gather(
                    xg[:, 0:Jc0, :], xrowg.ap()[0:HALF, :],
                    idx_t[:, 0:NI0 // 16], NI0, NI0, ELEM)
                nc.gpsimd.dma_gather(
                    xg[:, Jc0:Jc0 + Jc1, :], xrowg.ap()[HALF:NPAD, :],
                    idx_t[:, NI0 // 16:niw], NI1, NI1, ELEM)

                xb = sb.tile([128, WC], f16, tag="xb")
                nc.sync.dma_start(out=xb[:], in_=xbs.ap()[i * P:(i + 1) * P, :])

                # one-hot * w_hat build: eq per (h, s) range, one mult
                eq = sb.tile([128, JXmax, 32], f16, tag="eq")
                ci0 = 0
                for h in range(2):
                    e0 = 0
                    for s in range(4):
                        e1 = e0 + int(M[i, h, s])
                        nch = -(-e1 // 128) - e0 // 128
                        nc.vector.tensor_tensor(
                            out=eq[:, ci0:ci0 + nch, :],
                            in0=dmw_t[:, ci0:ci0 + nch].unsqueeze(2)
                                .to_broadcast([128, nch, 32]),
                            in1=iota_t[:, 32 * s:32 * s + 32].unsqueeze(1)
                                .to_broadcast([128, nch, 32]),
                            op=mybir.AluOpType.is_equal)
                        ci0 += nch
                        e0 = e1
                assert ci0 == jx, (ci0, jx)
                wm = sb.tile([128, JXmax, 32], f16, tag="wm")
                nc.vector.tensor_tensor(
                    out=wm[:, :jx, :],
                    in0=eq[:, :jx, :],
                    in1=dmw_t[:, jx:2 * jx].unsqueeze(2)
                        .to_broadcast([128, jx, 32]),
                    op=mybir.AluOpType.mult)

                # swapped reduce: T1^T [feat, dst] accumulated in PSUM
                psum_t1 = pst1.tile([128, 3, 128], f32, space="PSUM", tag="t1")
                for fc in range(3):
                    for (ci, j, s, st, sp) in streams[i]:
                        nc.tensor.matmul(
                            out=psum_t1[:, fc, 32 * s:32 * s + 32],
                            lhsT=xg[:, j, 128 * fc:128 * fc + 128],
                            rhs=wm[:, ci, :],
                            start=st, stop=sp)
                # T0^T via identity matmuls
                psum_t0 = pst0.tile([128, 3, 128], f32, space="PSUM", tag="t0")
                for fc in range(3):
                    nc.tensor.matmul(
                        out=psum_t0[:, fc, :],
                        lhsT=xb[:, 128 * fc:128 * fc + 128],
                        rhs=id_t[:],
                        start=True, stop=True)

                t1s = sb.tile([128, 3, 128], f16, tag="t1s")
                nc.scalar.copy(out=t1s[:], in_=psum_t1[:])
                t0s = sb.tile([128, 3, 128], f16, tag="t0s")
                nc.scalar.copy(out=t0s[:], in_=psum_t0[:])

                half = i % 2
                if half == 0:
                    osb = osbp.tile([128, 3, 256], f16, tag="osb")
                for go in range(3):
                    py = psy.tile([128, 128], f32, space="PSUM", tag="y")
                    plist = pairs_by_go[go]
                    for n_, (pi_, gi, path) in enumerate(plist):
                        rhs = (t0s if path == 0 else t1s)[:, gi, :]
                        nc.tensor.matmul(
                            out=py[:],
                            lhsT=mats_t[:, 128 * pi_:128 * pi_ + 128],
                            rhs=rhs,
                            start=(n_ == 0), stop=(n_ == len(plist) - 1))
                    ysl = osb[:, go, 128 * half:128 * half + 128]
                    nc.scalar.activation(
                        out=ysl, in_=py[:],
                        func=mybir.ActivationFunctionType.Identity,
                        bias=bias_t[:, go:go + 1], scale=1.0)
                oslice = osb[:, :, 128 * half:128 * half + 128]
                tl = sb.tile([128, 3, 128], f16, tag="tl")
                nc.vector.tensor_scalar_mul(out=tl[:], in0=oslice, scalar1=0.01)
                nc.vector.tensor_tensor(out=oslice, in0=oslice, in1=tl[:],
                                        op=mybir.AluOpType.max)
                if half == 1 or i == SLOTS - 1:
                    lo = (i - half) * P
                    nc.sync.dma_start(
                        out=out_pc.ap()[:, :, lo:lo + (half + 1) * P],
                        in_=osb[:, :, :(half + 1) * P])

    nc.compile()
    return nc


def kernel(x, A, Ew, Wcheb, bcheb, Wconv, bconv, batch_size=1):
    from concourse.bass_utils import run_bass_kernel_spmd

    prep = _host_prep(x, A, Ew)
    plan = prep["plan"]
    mats_sb, bias_sb, pairs = _fold_weights(Wcheb, bcheb, Wconv, bconv)

    key = (G8, prep["IWT"], prep["JXT"], tuple(plan["JX"].tolist()),
           tuple(plan["NI"].reshape(-1).tolist()))
    if key not in _cache:
        _cache[key] = _build_program(plan, prep["IWT"], prep["JXT"], len(pairs))
    nc = _cache[key]

    iota_np = np.tile(np.arange(128, dtype=np.float16)[None, :], (128, 1))
    ident_np = np.eye(128, dtype=np.float16)
    xg_src = prep["xrow8"] if G8 else prep["xrow16"]
    in_maps = []
    for c in range(NCORES):
        in_maps.append(dict(
            xrowg=xg_src, xbs=prep["xbs"][c], idx16=prep["idx16"][c],
            dmwh=prep["dmwh"][c], mats=mats_sb, biasd=bias_sb,
            iota4=iota_np, ident=ident_np))
    res = run_bass_kernel_spmd(nc, in_maps, core_ids=list(range(NCORES)))

    Bmap = plan["Bmap"]
    out = np.zeros((NPAD, W, C), np.float32)
    for c in range(NCORES):
        arr = np.asarray(res.results[c]["out_pc"], np.float32)  # [128,3,S*128]
        for i in range(SLOTS):
            b = int(Bmap[c, i])
            seg = arr[:, :, i * P:(i + 1) * P]          # [128(fo), 3(go), 128]
            blkout = seg.reshape(4, 32, 3, P).transpose(3, 2, 0, 1)
            out[b * P:(b + 1) * P] = blkout.reshape(P, W, C)
    return np.ascontiguousarray(out[:N])
